# revision 1
# baseline (speedup 1.0000x reference)
"""Multi-head attention (S=2048, B=2, D=1024, H=16, Hd=64) on 8 trn2 cores.

Sharding: core = (batch b, head-group g of 4 heads)  -> 2*4 = 8 cores.
Each core computes the full attention for its 4 heads / 1 batch and a
partial output projection (row-parallel Wo); the host sums the 4 partials
per batch and adds bo.

Per-core device kernel layout choices:
  - host pre-transposes x -> x^T [D, S] (bf16) so projections contract over
    D on partitions with no on-chip transposes.
  - q^T/k^T are produced pair-packed: [128(e of 2 heads), 2(pair), S] bf16.
  - v is stored per head as v_aug [t, 65] bf16 with column 64 = 1.0; the
    attention matmul (M=65) then yields row 64 = softmax denominator Z.
  - scores are computed transposed ([t, s]) with K=64 row-tiled matmul
    pairs (two heads concurrently in PE row groups 0 and 2).
  - exp on ACT (psum -> sbuf bf16, scale=1/8); softmax normalization via
    reciprocal_approx_fast + DRAM-bounce partition broadcast.
  - output projection in float32r (full-rate fp32) accumulating 4 heads
    per PSUM tile.
"""

import sys

for _p in ("/opt/trn_rl_repo", "/root/.axon_site/_ro/trn_rl_repo"):
    if _p not in sys.path:
        sys.path.insert(0, _p)

import numpy as np
import ml_dtypes

S = 2048
B = 2
D = 1024
H = 16
HD = 64
NH = 4  # heads per core
P = 128
KD = D // P  # 8 contraction tiles for projections

BF16 = ml_dtypes.bfloat16

_BUILD_CACHE = {}


def build_bass(s=S, debug_taps=False):
    """Build the per-core Bass module (same program for all 8 cores)."""
    import concourse.bacc as bacc
    import concourse.bass as bass
    import concourse.mybir as mybir
    import concourse.tile as tile

    f32 = mybir.dt.float32
    f32r = mybir.dt.float32r
    bf16 = mybir.dt.bfloat16
    AF = mybir.ActivationFunctionType
    ALU = mybir.AluOpType

    NT = s // P            # t tiles
    WSC = min(1024, s)     # scores/exp tile width (s columns)
    NSH = s // WSC         # s-half rounds
    CW = min(512, WSC)     # chain width (one psum bank)
    NCH = WSC // CW        # chains per head per round

    nc = bacc.Bacc("TRN2", target_bir_lowering=False, debug=False, num_devices=8)

    xq = nc.dram_tensor("xq_t", [D, s], bf16, kind="ExternalInput").ap()
    xk = nc.dram_tensor("xk_t", [D, s], bf16, kind="ExternalInput").ap()
    xv = nc.dram_tensor("xv_t", [D, s], bf16, kind="ExternalInput").ap()
    wq = nc.dram_tensor("wq_t", [D, 256], bf16, kind="ExternalInput").ap()
    wk = nc.dram_tensor("wk_t", [D, 256], bf16, kind="ExternalInput").ap()
    wv = nc.dram_tensor("wv_t", [D, 256], bf16, kind="ExternalInput").ap()
    wo = nc.dram_tensor("wo_h", [P, 2, D], f32r, kind="ExternalInput").ap()
    bq2 = nc.dram_tensor("bq2", [P, 2], f32, kind="ExternalInput").ap()
    bk2 = nc.dram_tensor("bk2", [P, 2], f32, kind="ExternalInput").ap()
    bv4 = nc.dram_tensor("bv4", [P, 256], f32, kind="ExternalInput").ap()
    out = nc.dram_tensor("out", [s, D], f32, kind="ExternalOutput").ap()

    from contextlib import ExitStack

    with tile.TileContext(nc) as tc, ExitStack() as ctx:
        consts = ctx.enter_context(tc.tile_pool(name="consts", bufs=1))
        persist = ctx.enter_context(tc.tile_pool(name="persist", bufs=1))
        xpool = ctx.enter_context(tc.tile_pool(name="xpool", bufs=2 * KD))
        epool = ctx.enter_context(tc.tile_pool(name="epool", bufs=16))
        rzpool = ctx.enter_context(tc.tile_pool(name="rzpool", bufs=2))
        ospool = ctx.enter_context(tc.tile_pool(name="ospool", bufs=3))
        drampool = ctx.enter_context(tc.tile_pool(name="drampool", bufs=2, space="DRAM"))
        wide = ctx.enter_context(tc.tile_pool(name="wide", bufs=2, space="PSUM"))
        accp = ctx.enter_context(tc.tile_pool(name="accp", bufs=2 * NCH, space="PSUM"))

        # ---- constants -------------------------------------------------
        wq_sb = consts.tile([P, KD, 256], bf16, name="wq_sb")
        nc.sync.dma_start(out=wq_sb, in_=wq.rearrange("(k p) e -> p k e", p=P))
        wk_sb = consts.tile([P, KD, 256], bf16, name="wk_sb")
        nc.sync.dma_start(out=wk_sb, in_=wk.rearrange("(k p) e -> p k e", p=P))
        wv_sb = consts.tile([P, KD, 256], bf16, name="wv_sb")
        nc.sync.dma_start(out=wv_sb, in_=wv.rearrange("(k p) e -> p k e", p=P))
        wo_sb = consts.tile([P, 2, D], f32r, name="wo_sb")
        nc.sync.dma_start(out=wo_sb, in_=wo)
        bq_sb = consts.tile([P, 2], f32, name="bq_sb")
        nc.sync.dma_start(out=bq_sb, in_=bq2)
        bk_sb = consts.tile([P, 2], f32, name="bk_sb")
        nc.sync.dma_start(out=bk_sb, in_=bk2)
        bv_sb = consts.tile([P, 256], f32, name="bv_sb")
        nc.sync.dma_start(out=bv_sb, in_=bv4)

        # ---- persistent activations -----------------------------------
        q2 = persist.tile([P, 2, s], bf16, name="q2")
        k2 = persist.tile([P, 2, s], bf16, name="k2")
        v_aug = persist.tile([P, NH, NT, 65], bf16, name="v_aug")
        nc.vector.memset(v_aug, 1.0)  # col 64 stays 1.0 = Z ones column
        # attn2: pair-packed normalized attention [128(e of 2 heads), 2, s]
        attn2 = persist.tile([P, 2, s], f32r, name="attn2")

        # ---- load x^T and project -------------------------------------
        def load_x(xdram):
            x3 = xdram.rearrange("(k p) s -> k p s", p=P)
            tiles = []
            for k in range(KD):
                xt = xpool.tile([P, s], bf16, tag="x", name=f"xt{k}")
                nc.sync.dma_start(out=xt, in_=x3[k])
                tiles.append(xt)
            return tiles

        def proj_round(xtiles, w_sb, b_sb, dst, p, sh):
            # dst[:, p, sh-slice] = ((x @ W_pair.T)^T + bias) for one s-half
            ps = wide.tile([P, WSC], f32, tag="wide", name="qkps")
            for c in range(NCH):
                for k in range(KD):
                    nc.tensor.matmul(
                        ps[:, c * CW:(c + 1) * CW],
                        lhsT=w_sb[:, k, p * P:(p + 1) * P],
                        rhs=xtiles[k][:, sh * WSC + c * CW: sh * WSC + (c + 1) * CW],
                        start=(k == 0),
                        stop=(k == KD - 1),
                    )
            nc.vector.tensor_scalar(
                dst[:, p, sh * WSC:(sh + 1) * WSC], ps, b_sb[:, p:p + 1],
                None, ALU.add,
            )

        def v_round(xtiles, t):
            ps = wide.tile([P, 256], f32, tag="wide", name="vps")
            for k in range(KD):
                nc.tensor.matmul(
                    ps,
                    lhsT=xtiles[k][:, t * P:(t + 1) * P],
                    rhs=wv_sb[:, k, :],
                    start=(k == 0),
                    stop=(k == KD - 1),
                )
            for h in range(NH):
                nc.vector.tensor_tensor(
                    v_aug[:, h, t, 0:64],
                    ps[:, h * 64:(h + 1) * 64],
                    bv_sb[:, h * 64:(h + 1) * 64],
                    ALU.add,
                )

        xq_tiles = load_x(xq)
        for p in range(2):
            for sh in range(NSH):
                proj_round(xq_tiles, wq_sb, bq_sb, q2, p, sh)
        xk_tiles = load_x(xk)
        for p in range(2):
            for sh in range(NSH):
                proj_round(xk_tiles, wk_sb, bk_sb, k2, p, sh)
        xv_tiles = load_x(xv)
        for t in range(NT):
            v_round(xv_tiles, t)

        def out_proj(sc_i):
            op = wide.tile([P, D], f32, tag="wide", name="op")
            for nh_i in range(2):
                for p in range(2):
                    nc.tensor.matmul(
                        op[:, nh_i * 512:(nh_i + 1) * 512],
                        lhsT=attn2[:, p, sc_i * P:(sc_i + 1) * P],
                        rhs=wo_sb[:, p, nh_i * 512:(nh_i + 1) * 512],
                        start=(p == 0),
                        stop=(p == 1),
                    )
            ob = ospool.tile([P, D], f32, tag="ob", name="ob")
            nc.vector.tensor_copy(ob, op)
            nc.sync.dma_start(out=out[sc_i * P:(sc_i + 1) * P, :], in_=ob)

        def normalize(p, hi, soff, chains):
            # attn = attn~ / Z ; Z sits in row 64 of each chain
            rz = rzpool.tile([P, WSC], f32, tag="rz", name="rz")
            for c in range(NCH):
                nc.vector.tensor_copy(
                    rz[64:65, c * CW:(c + 1) * CW],
                    chains[c][64:65, :],
                )
            zd = drampool.tile([1, WSC], f32, tag="zd", name="zd")
            nc.sync.dma_start(out=zd, in_=rz[64:65, :])
            zbc = bass.AP(
                tensor=zd.tensor,
                offset=zd.offset,
                ap=[[0, 64]] + list(zd.ap[-1:]),
            )
            nc.sync.dma_start(out=rz[0:64, :], in_=zbc)
            # reciprocal at base partition 0 (base 64 miscomputes on HW)
            nc.vector.reciprocal_approx_fast(rz[0:64, :], rz[0:64, :])
            if hi == 0:
                # even head of pair -> attn2 rows 0:64 directly
                for c in range(NCH):
                    nc.vector.tensor_tensor(
                        attn2[0:64, p, soff + c * CW: soff + (c + 1) * CW],
                        chains[c][0:64, :],
                        rz[0:64, c * CW:(c + 1) * CW],
                        ALU.mult,
                    )
            else:
                # odd head: drain to tmp then DMA-shift to rows 64:128
                atmp = rzpool.tile([HD, WSC], f32r, tag="atmp", name="atmp")
                for c in range(NCH):
                    nc.vector.tensor_tensor(
                        atmp[:, c * CW:(c + 1) * CW],
                        chains[c][0:64, :],
                        rz[0:64, c * CW:(c + 1) * CW],
                        ALU.mult,
                    )
                nc.sync.dma_start(
                    out=attn2[64:128, p, soff:soff + WSC], in_=atmp
                )

        for sh in range(NSH):
            soff = sh * WSC
            for p in range(2):
                heads = (2 * p, 2 * p + 1)
                chains = [
                    [accp.tile([P, CW], f32, tag="chain", name=f"ch{hi}_{c}")
                     for c in range(NCH)]
                    for hi in range(2)
                ]
                for t in range(NT):
                    etiles = []
                    for hi in range(2):
                        rlo, rhi = (0, 64) if hi == 0 else (64, 128)
                        sc = wide.tile([P, WSC], f32, tag="wide", name=f"sc{hi}")
                        for c in range(NCH):
                            nc.tensor.matmul(
                                sc[:, c * CW:(c + 1) * CW],
                                lhsT=k2[rlo:rhi, p, t * P:(t + 1) * P],
                                rhs=q2[rlo:rhi, p, soff + c * CW: soff + (c + 1) * CW],
                                start=True,
                                stop=True,
                                tile_position=(rlo, 0),
                            )
                        et = epool.tile([P, WSC], bf16, tag="exp", name=f"exp{hi}")
                        nc.scalar.activation(et, sc, AF.Exp, bias=0.0, scale=0.125)
                        etiles.append(et)
                    for hi in range(2):
                        for c in range(NCH):
                            nc.tensor.matmul(
                                chains[hi][c][0:65, :],
                                lhsT=v_aug[:, heads[hi], t, :],
                                rhs=etiles[hi][:, c * CW:(c + 1) * CW],
                                start=(t == 0),
                                stop=(t == NT - 1),
                            )
                normalize(p, 0, soff, chains[0])
                normalize(p, 1, soff, chains[1])

        for sc_i in range(s // P):
            out_proj(sc_i)

        if debug_taps:
            dq2 = nc.dram_tensor("dbg_q2", [P, 2, s], bf16, kind="ExternalOutput").ap()
            nc.sync.dma_start(out=dq2, in_=q2)
            dk2 = nc.dram_tensor("dbg_k2", [P, 2, s], bf16, kind="ExternalOutput").ap()
            nc.sync.dma_start(out=dk2, in_=k2)
            dva = nc.dram_tensor("dbg_vaug", [P, NH, NT, 65], bf16, kind="ExternalOutput").ap()
            nc.sync.dma_start(out=dva, in_=v_aug)
            dat = nc.dram_tensor("dbg_attn", [P, 2, s], f32, kind="ExternalOutput").ap()
            nc.sync.dma_start(out=dat, in_=attn2.bitcast(f32))

    nc.compile()
    return nc


def get_bass(s=S):
    if s not in _BUILD_CACHE:
        _BUILD_CACHE[s] = build_bass(s)
    return _BUILD_CACHE[s]


def make_in_maps(query, key, value, Wq, bq, Wk, bk, Wv, bv, Wo):
    """Host-side sharding: per-core input dict for core = b*4 + g."""
    in_maps = []
    for core in range(8):
        b, g = core // 4, core % 4
        cs = slice(g * 256, (g + 1) * 256)
        # pair-packed: wo_h[hd + 64*(h%2), h//2, :] = Wo[:, g*256 + h*64 + hd]
        wo_h = (
            np.ascontiguousarray(Wo[:, cs].T)  # [256(h*64+hd), 1024]
            .reshape(2, P, D)
            .transpose(1, 0, 2)
        )
        m = {
            "xq_t": np.ascontiguousarray(query[:, b, :].T).astype(BF16),
            "xk_t": np.ascontiguousarray(key[:, b, :].T).astype(BF16),
            "xv_t": np.ascontiguousarray(value[:, b, :].T).astype(BF16),
            "wq_t": np.ascontiguousarray(Wq[cs, :].T).astype(BF16),
            "wk_t": np.ascontiguousarray(Wk[cs, :].T).astype(BF16),
            "wv_t": np.ascontiguousarray(Wv[cs, :].T).astype(BF16),
            "wo_h": np.ascontiguousarray(wo_h).astype(np.float32),
            "bq2": np.ascontiguousarray(bq[cs].reshape(2, P).T).astype(np.float32),
            "bk2": np.ascontiguousarray(bk[cs].reshape(2, P).T).astype(np.float32),
            "bv4": np.ascontiguousarray(
                np.broadcast_to(bv[cs], (P, 256))
            ).astype(np.float32),
        }
        in_maps.append(m)
    return in_maps


def kernel(query, key, value, Wq, bq, Wk, bk, Wv, bv, Wo, bo):
    from concourse.bass_utils import run_bass_kernel_spmd

    query = np.asarray(query, dtype=np.float32)
    key = np.asarray(key, dtype=np.float32)
    value = np.asarray(value, dtype=np.float32)
    Wq = np.asarray(Wq, dtype=np.float32)
    Wk = np.asarray(Wk, dtype=np.float32)
    Wv = np.asarray(Wv, dtype=np.float32)
    Wo = np.asarray(Wo, dtype=np.float32)

    nc = get_bass(S)
    in_maps = make_in_maps(query, key, value, Wq, bq, Wk, bk, Wv, bv, Wo)
    res = run_bass_kernel_spmd(nc, in_maps, core_ids=list(range(8)))
    outs = [res.results[c]["out"] for c in range(8)]

    full = np.empty((S, B, D), dtype=np.float32)
    bo32 = np.asarray(bo, dtype=np.float32)
    for b in range(B):
        acc = outs[b * 4].astype(np.float32).copy()
        for g in range(1, 4):
            acc += outs[b * 4 + g]
        full[:, b, :] = acc + bo32[None, :]
    return full



# revision 3
# speedup vs baseline: 1.0019x; 1.0019x over previous
"""Multi-head attention (S=2048, B=2, D=1024, H=16, Hd=64) on 8 trn2 cores.

Sharding: core = (batch b, head-group g of 4 heads)  -> 2*4 = 8 cores.
Each core computes the full attention for its 4 heads / 1 batch and a
partial output projection (row-parallel Wo); the host sums the 4 partials
per batch and adds bo.

Schedule (v2): software-pipelined around the ACT engine's exp wall.
  - 8 attention rounds of (sh in 4 s-blocks of 512, p in 2 head-pairs);
    per t-step the PE does 2 score mms (row-paired heads at tile_position
    0/64) + 2 attn chain mms; ACT does one exp over [128, 1024] (both
    heads packed side by side in one PSUM score tile).
  - PSUM: scores 2x[128,1024] (4 banks) + chains 2x[128,512] (2 banks)
    + fill pool 2x[128,512] (2 banks) for proj/out-proj work that is
    interleaved into the rounds as PE filler (keeps the PE p-state up).
  - x DMAs are ordered xk -> xv -> xq (xq in 4 column quarters) so the
    k-projection starts ~1.5us in and round 0 starts as soon as
    q2[sh0] lands.
  - softmax Z broadcast via gpsimd partition_broadcast (no DRAM bounce);
    odd head of each pair reaches attn2 rows 64:128 via an SBUF->SBUF
    DMA partition shift.
  - out-proj is chunked per 128 output rows and interleaved as filler;
    each chunk DMAs out immediately.
"""

import sys

for _p in ("/opt/trn_rl_repo", "/root/.axon_site/_ro/trn_rl_repo"):
    if _p not in sys.path:
        sys.path.insert(0, _p)

import numpy as np
import ml_dtypes

S = 2048
B = 2
D = 1024
H = 16
HD = 64
NH = 4  # heads per core
P = 128
KD = D // P  # 8 contraction tiles for projections
NT = S // P  # 16 t tiles
WSC = 512  # s-columns per round
NSH = S // WSC  # 4 s-blocks
NCH = S // P  # out-proj chunks (16)

BF16 = ml_dtypes.bfloat16

_BUILD_CACHE = {}


def build_bass(s=S):
    """Build the per-core Bass module (same program for all 8 cores)."""
    import concourse.bacc as bacc
    import concourse.bass as bass
    import concourse.mybir as mybir
    import concourse.tile as tile

    f32 = mybir.dt.float32
    f32r = mybir.dt.float32r
    bf16 = mybir.dt.bfloat16
    AF = mybir.ActivationFunctionType
    ALU = mybir.AluOpType

    nc = bacc.Bacc("TRN2", target_bir_lowering=False, debug=False, num_devices=8)

    xq = nc.dram_tensor("xq_t", [D, s], bf16, kind="ExternalInput").ap()
    xk = nc.dram_tensor("xk_t", [D, s], bf16, kind="ExternalInput").ap()
    xv = nc.dram_tensor("xv_t", [D, s], bf16, kind="ExternalInput").ap()
    wq = nc.dram_tensor("wq_t", [D, 256], bf16, kind="ExternalInput").ap()
    wk = nc.dram_tensor("wk_t", [D, 256], bf16, kind="ExternalInput").ap()
    wv = nc.dram_tensor("wv_t", [D, 256], bf16, kind="ExternalInput").ap()
    wo = nc.dram_tensor("wo_h", [P, 2, D], f32r, kind="ExternalInput").ap()
    bq2 = nc.dram_tensor("bq2", [P, 2], f32, kind="ExternalInput").ap()
    bk2 = nc.dram_tensor("bk2", [P, 2], f32, kind="ExternalInput").ap()
    bv4 = nc.dram_tensor("bv4", [P, 256], f32, kind="ExternalInput").ap()
    out = nc.dram_tensor("out", [s, D], f32, kind="ExternalOutput").ap()

    from contextlib import ExitStack

    with tile.TileContext(nc) as tc, ExitStack() as ctx:
        consts = ctx.enter_context(tc.tile_pool(name="consts", bufs=1))
        persist = ctx.enter_context(tc.tile_pool(name="persist", bufs=1))
        xkpool = ctx.enter_context(tc.tile_pool(name="xkpool", bufs=KD))
        xvpool = ctx.enter_context(tc.tile_pool(name="xvpool", bufs=KD))
        xqpool = ctx.enter_context(tc.tile_pool(name="xqpool", bufs=2 * KD))
        epool = ctx.enter_context(tc.tile_pool(name="epool", bufs=4))
        rzpool = ctx.enter_context(tc.tile_pool(name="rzpool", bufs=2))
        ospool = ctx.enter_context(tc.tile_pool(name="ospool", bufs=3))
        scp = ctx.enter_context(tc.tile_pool(name="scp", bufs=2, space="PSUM"))
        chp = ctx.enter_context(tc.tile_pool(name="chp", bufs=2, space="PSUM"))
        fillp = ctx.enter_context(tc.tile_pool(name="fillp", bufs=2, space="PSUM"))

        # ---- constants (small, queued first) --------------------------
        wq_sb = consts.tile([P, KD, 256], bf16, name="wq_sb")
        nc.sync.dma_start(out=wq_sb, in_=wq.rearrange("(k p) e -> p k e", p=P))
        wk_sb = consts.tile([P, KD, 256], bf16, name="wk_sb")
        nc.sync.dma_start(out=wk_sb, in_=wk.rearrange("(k p) e -> p k e", p=P))
        wv_sb = consts.tile([P, KD, 256], bf16, name="wv_sb")
        nc.sync.dma_start(out=wv_sb, in_=wv.rearrange("(k p) e -> p k e", p=P))
        wo_sb = consts.tile([P, 2, D], f32r, name="wo_sb")
        nc.sync.dma_start(out=wo_sb, in_=wo)
        bq_sb = consts.tile([P, 2], f32, name="bq_sb")
        nc.sync.dma_start(out=bq_sb, in_=bq2)
        bk_sb = consts.tile([P, 2], f32, name="bk_sb")
        nc.sync.dma_start(out=bk_sb, in_=bk2)
        bv_sb = consts.tile([P, 256], f32, name="bv_sb")
        nc.sync.dma_start(out=bv_sb, in_=bv4)

        # ---- persistent activations -----------------------------------
        q2 = persist.tile([P, 2, s], bf16, name="q2")
        k2 = persist.tile([P, 2, s], bf16, name="k2")
        v_aug = persist.tile([P, NH, NT, 65], bf16, name="v_aug")
        nc.vector.memset(v_aug, 1.0)  # col 64 stays 1.0 = Z ones column
        # attn2: pair-packed normalized attention [128(e of 2 heads), 2, s]
        attn2 = persist.tile([P, 2, s], f32r, name="attn2")

        # ---- x DMAs, priority order: xk, xv, xq quarters --------------
        xk3 = xk.rearrange("(k p) s -> k p s", p=P)
        xk_tiles = []
        for k in range(KD):
            t_ = xkpool.tile([P, s], bf16, tag="xk", name=f"xk{k}")
            nc.sync.dma_start(out=t_, in_=xk3[k])
            xk_tiles.append(t_)
        xv3 = xv.rearrange("(k p) s -> k p s", p=P)
        xv_tiles = []
        for k in range(KD):
            t_ = xvpool.tile([P, s], bf16, tag="xv", name=f"xv{k}")
            nc.sync.dma_start(out=t_, in_=xv3[k])
            xv_tiles.append(t_)
        xq3 = xq.rearrange("(k p) s -> k p s", p=P)
        xq_tiles = {}  # (k, sh) -> [P, WSC] tile
        for sh in range(NSH):
            for k in range(KD):
                t_ = xqpool.tile([P, WSC], bf16, tag="xq", name=f"xq{k}_{sh}")
                nc.sync.dma_start(
                    out=t_, in_=xq3[k][:, sh * WSC:(sh + 1) * WSC]
                )
                xq_tiles[(k, sh)] = t_

        # ---- projection helpers (run in the fill PSUM pool) -----------
        def qk_proj(xget, w_sb, b_sb, dst, p, sh):
            # dst[:, p, sh-block] = (x @ W_pair.T)^T + bias  for 512 cols
            ps = fillp.tile([P, WSC], f32, tag="fill", name="qkps")
            for k in range(KD):
                nc.tensor.matmul(
                    ps,
                    lhsT=w_sb[:, k, p * P:(p + 1) * P],
                    rhs=xget(k, sh),
                    start=(k == 0),
                    stop=(k == KD - 1),
                )
            nc.vector.tensor_scalar(
                dst[:, p, sh * WSC:(sh + 1) * WSC], ps, b_sb[:, p:p + 1],
                None, ALU.add,
            )

        def v_proj(t):
            ps = fillp.tile([P, WSC], f32, tag="fill", name="vps")
            for k in range(KD):
                nc.tensor.matmul(
                    ps[:, 0:256],
                    lhsT=xv_tiles[k][:, t * P:(t + 1) * P],
                    rhs=wv_sb[:, k, :],
                    start=(k == 0),
                    stop=(k == KD - 1),
                )
            for h in range(NH):
                nc.vector.tensor_tensor(
                    v_aug[:, h, t, 0:64],
                    ps[:, h * 64:(h + 1) * 64],
                    bv_sb[:, h * 64:(h + 1) * 64],
                    ALU.add,
                )

        def out_chunk(ci):
            # out rows [ci*128, (ci+1)*128) ; contract attn2 over both pairs
            ob = ospool.tile([P, D], f32, tag="ob", name="ob")
            for nh_i in range(2):
                op = fillp.tile([P, WSC], f32, tag="fill", name="op")
                for p in range(2):
                    nc.tensor.matmul(
                        op,
                        lhsT=attn2[:, p, ci * P:(ci + 1) * P],
                        rhs=wo_sb[:, p, nh_i * 512:(nh_i + 1) * 512],
                        start=(p == 0),
                        stop=(p == 1),
                    )
                nc.vector.tensor_copy(ob[:, nh_i * 512:(nh_i + 1) * 512], op)
            nc.sync.dma_start(out=out[ci * P:(ci + 1) * P, :], in_=ob)

        def normalize(p, sh, ch0, ch1):
            soff = sh * WSC
            rz = rzpool.tile([P, 2, WSC], f32, tag="rz", name="rz")
            nc.vector.tensor_copy(rz[64:65, 0, :], ch0[64:65, :])
            nc.vector.tensor_copy(rz[64:65, 1, :], ch1[64:65, :])
            # partition_broadcast only reads partition 0 correctly; DMA-shift
            # the Z row from partition 64 down to a partition-0 tile first.
            z0 = rzpool.tile([1, 2, WSC], f32, tag="z0", name="z0")
            nc.sync.dma_start(out=z0, in_=rz[64:65])
            nc.gpsimd.partition_broadcast(rz[0:64], z0)
            # reciprocal at base partition 0 (base 64 miscomputes on HW)
            nc.vector.reciprocal_approx_fast(rz[0:64], rz[0:64])
            # even head of pair -> attn2 rows 0:64 directly
            nc.vector.tensor_tensor(
                attn2[0:64, p, soff:soff + WSC],
                ch0[0:64, :],
                rz[0:64, 0, :],
                ALU.mult,
            )
            # odd head: drain to tmp then DMA-shift to rows 64:128
            atmp = rzpool.tile([HD, WSC], f32r, tag="atmp", name="atmp")
            nc.vector.tensor_tensor(
                atmp, ch1[0:64, :], rz[0:64, 1, :], ALU.mult,
            )
            nc.sync.dma_start(out=attn2[64:128, p, soff:soff + WSC], in_=atmp)

        # ---- lead-in: k-proj (k-DMA paced), v(0..7), q(sh0) -----------
        for p in range(2):
            for sh in range(NSH):
                qk_proj(
                    lambda k, sh_: xk_tiles[k][:, sh_ * WSC:(sh_ + 1) * WSC],
                    wk_sb, bk_sb, k2, p, sh,
                )
        for t in range(NT // 2):
            v_proj(t)
        for p in range(2):
            qk_proj(lambda k, sh_: xq_tiles[(k, sh_)], wq_sb, bq_sb, q2, p, 0)

        # ---- filler schedule per round --------------------------------
        # round r = sh*2 + p ; out-proj for sh becomes available during
        # round sh*2+2 (after both p-normalizes).
        fillers = {r: [] for r in range(2 * NSH)}
        for i, t in enumerate(range(NT // 2, NT)):
            # forced into round 0 at a slot strictly before t (attn needs it)
            fillers[0].append((min(2 * i, t - 1), lambda t=t: v_proj(t)))
        qjobs = [(sh, p) for sh in range(1, NSH) for p in range(2)]
        qslots = {1: [1, 8], 2: [12], 3: [4, 12], 4: [8]}
        qi = 0
        for r, slots in qslots.items():
            for sl in slots:
                sh, p = qjobs[qi]
                qi += 1
                fillers[r].append(
                    (sl, lambda sh=sh, p=p: qk_proj(
                        lambda k, sh_: xq_tiles[(k, sh_)], wq_sb, bq_sb, q2, p, sh))
                )
        assert qi == len(qjobs)
        # out-proj chunks: sh's 4 chunks spread over rounds sh*2+2, +3
        for sh in range(NSH - 1):
            for j in range(4):
                r = sh * 2 + 2 + j // 2
                sl = 2 + 6 * (j % 2) + 3
                fillers[r].append((sl, lambda ci=sh * 4 + j: out_chunk(ci)))

        # ---- attention rounds -----------------------------------------
        for r in range(2 * NSH):
            sh, p = r // 2, r % 2
            soff = sh * WSC
            heads = (2 * p, 2 * p + 1)
            ch0 = chp.tile([P, WSC], f32, tag="ch", name="ch0")
            ch1 = chp.tile([P, WSC], f32, tag="ch", name="ch1")
            byslot = {}
            for sl, job in fillers[r]:
                byslot.setdefault(sl, []).append(job)
            for t in range(NT):
                sc = scp.tile([P, 2 * WSC], f32, tag="sc", name="sc")
                for hi in range(2):
                    rlo, rhi = (0, 64) if hi == 0 else (64, 128)
                    nc.tensor.matmul(
                        sc[:, hi * WSC:(hi + 1) * WSC],
                        lhsT=k2[rlo:rhi, p, t * P:(t + 1) * P],
                        rhs=q2[rlo:rhi, p, soff:soff + WSC],
                        start=True,
                        stop=True,
                        tile_position=(rlo, 0),
                    )
                et = epool.tile([P, 2 * WSC], bf16, tag="exp", name="et")
                nc.scalar.activation(et, sc, AF.Exp, bias=0.0, scale=0.125)
                nc.tensor.matmul(
                    ch0[0:65, :],
                    lhsT=v_aug[:, heads[0], t, :],
                    rhs=et[:, 0:WSC],
                    start=(t == 0),
                    stop=(t == NT - 1),
                )
                nc.tensor.matmul(
                    ch1[0:65, :],
                    lhsT=v_aug[:, heads[1], t, :],
                    rhs=et[:, WSC:2 * WSC],
                    start=(t == 0),
                    stop=(t == NT - 1),
                )
                for job in byslot.get(t, []):
                    job()
            normalize(p, sh, ch0, ch1)

        # ---- tail: last sh out-proj chunks ----------------------------
        for j in range(4):
            out_chunk((NSH - 1) * 4 + j)

    nc.compile()
    return nc


def get_bass(s=S):
    if s not in _BUILD_CACHE:
        _BUILD_CACHE[s] = build_bass(s)
    return _BUILD_CACHE[s]


def make_in_maps(query, key, value, Wq, bq, Wk, bk, Wv, bv, Wo):
    """Host-side sharding: per-core input dict for core = b*4 + g."""
    in_maps = []
    for core in range(8):
        b, g = core // 4, core % 4
        cs = slice(g * 256, (g + 1) * 256)
        # pair-packed: wo_h[hd + 64*(h%2), h//2, :] = Wo[:, g*256 + h*64 + hd]
        wo_h = (
            np.ascontiguousarray(Wo[:, cs].T)  # [256(h*64+hd), 1024]
            .reshape(2, P, D)
            .transpose(1, 0, 2)
        )
        m = {
            "xq_t": np.ascontiguousarray(query[:, b, :].T).astype(BF16),
            "xk_t": np.ascontiguousarray(key[:, b, :].T).astype(BF16),
            "xv_t": np.ascontiguousarray(value[:, b, :].T).astype(BF16),
            "wq_t": np.ascontiguousarray(Wq[cs, :].T).astype(BF16),
            "wk_t": np.ascontiguousarray(Wk[cs, :].T).astype(BF16),
            "wv_t": np.ascontiguousarray(Wv[cs, :].T).astype(BF16),
            "wo_h": np.ascontiguousarray(wo_h).astype(np.float32),
            "bq2": np.ascontiguousarray(bq[cs].reshape(2, P).T).astype(np.float32),
            "bk2": np.ascontiguousarray(bk[cs].reshape(2, P).T).astype(np.float32),
            "bv4": np.ascontiguousarray(
                np.broadcast_to(bv[cs], (P, 256))
            ).astype(np.float32),
        }
        in_maps.append(m)
    return in_maps


def kernel(query, key, value, Wq, bq, Wk, bk, Wv, bv, Wo, bo):
    from concourse.bass_utils import run_bass_kernel_spmd

    query = np.asarray(query, dtype=np.float32)
    key = np.asarray(key, dtype=np.float32)
    value = np.asarray(value, dtype=np.float32)
    Wq = np.asarray(Wq, dtype=np.float32)
    Wk = np.asarray(Wk, dtype=np.float32)
    Wv = np.asarray(Wv, dtype=np.float32)
    Wo = np.asarray(Wo, dtype=np.float32)

    nc = get_bass(S)
    in_maps = make_in_maps(query, key, value, Wq, bq, Wk, bk, Wv, bv, Wo)
    res = run_bass_kernel_spmd(nc, in_maps, core_ids=list(range(8)))
    outs = [res.results[c]["out"] for c in range(8)]

    full = np.empty((S, B, D), dtype=np.float32)
    bo32 = np.asarray(bo, dtype=np.float32)
    for b in range(B):
        acc = outs[b * 4].astype(np.float32).copy()
        for g in range(1, 4):
            acc += outs[b * 4 + g]
        full[:, b, :] = acc + bo32[None, :]
    return full


# revision 9
# speedup vs baseline: 1.2111x; 1.2088x over previous
"""Multi-head attention (S=2048, B=2, D=1024, H=16, Hd=64) on 8 trn2 cores.

Sharding: core = (batch b, head-group g of 4 heads)  -> 2*4 = 8 cores.
Each core computes the full attention for its 4 heads / 1 batch and a
partial output projection (row-parallel Wo); the host sums the 4 partials
per batch and adds bo.

Schedule (v3): software-pipelined around the ACT engine's exp wall.
  - 8 attention rounds of (sh in 4 s-blocks of 512, p in 2 head-pairs);
    per t-step the PE does 2 score mms (row-paired heads at tile_position
    0/64) + 2 attn chain mms (emitted with lag 2 so chain-buffer reuse
    stalls never block the score stream); ACT does one exp over
    [128, 1024] (both heads packed side by side in one PSUM score tile).
  - PSUM: scores 2x[128,1024] (4 banks) + chains 2x[128,512] (2 banks)
    + fill pool 2x[128,512] (2 banks) for proj/out-proj work that is
    interleaved into the rounds as PE filler (keeps the PE p-state up).
  - DMA order: wk, xk, wq, xq[sh0], wv, xv, xq[sh1..3], wo - so the
    k-projection starts as soon as the first xk tile lands and round 0
    starts right after q2[sh0]; the v-projection runs as round-0 filler
    (attn lag 4 there so it never blocks the score stream).
  - normalize: chains are drained to SBUF immediately (frees the chain
    PSUM bank for the next round after one DVE copy); Z goes partition
    64 -> 0 via a small gpsimd-issued SBUF DMA, then gpsimd
    partition_broadcast (which only honors partition-0 sources), DVE
    reciprocal and the scaling multiplies - all off the PE critical path.
  - out-proj is chunked per 128 output rows and interleaved as filler;
    each chunk DMAs out immediately from the SP queue.
"""

import sys

for _p in ("/opt/trn_rl_repo", "/root/.axon_site/_ro/trn_rl_repo"):
    if _p not in sys.path:
        sys.path.insert(0, _p)

import numpy as np
import ml_dtypes

S = 2048
B = 2
D = 1024
H = 16
HD = 64
NH = 4  # heads per core
P = 128
KD = D // P  # 8 contraction tiles for projections
NT = S // P  # 16 t tiles
WSC = 512  # s-columns per round
NSH = S // WSC  # 4 s-blocks

BF16 = ml_dtypes.bfloat16

_BUILD_CACHE = {}


def build_bass(s=S):
    """Build the per-core Bass module (same program for all 8 cores)."""
    import concourse.bacc as bacc
    import concourse.bass as bass
    import concourse.mybir as mybir
    import concourse.tile as tile

    f32 = mybir.dt.float32
    f32r = mybir.dt.float32r
    bf16 = mybir.dt.bfloat16
    AF = mybir.ActivationFunctionType
    ALU = mybir.AluOpType

    nc = bacc.Bacc("TRN2", target_bir_lowering=False, debug=False, num_devices=8)

    xq = nc.dram_tensor("xq_t", [D, s], bf16, kind="ExternalInput").ap()
    xk = nc.dram_tensor("xk_t", [D, s], bf16, kind="ExternalInput").ap()
    xv = nc.dram_tensor("xv_t", [D, s], bf16, kind="ExternalInput").ap()
    wq = nc.dram_tensor("wq_t", [D, 256], bf16, kind="ExternalInput").ap()
    wk = nc.dram_tensor("wk_t", [D, 256], bf16, kind="ExternalInput").ap()
    wv = nc.dram_tensor("wv_t", [D, 256], bf16, kind="ExternalInput").ap()
    wo = nc.dram_tensor("wo_h", [P, 2, D], f32r, kind="ExternalInput").ap()
    bq2 = nc.dram_tensor("bq2", [P, 2], f32, kind="ExternalInput").ap()
    bk2 = nc.dram_tensor("bk2", [P, 2], f32, kind="ExternalInput").ap()
    bv4 = nc.dram_tensor("bv4", [P, 256], f32, kind="ExternalInput").ap()
    out = nc.dram_tensor("out", [s, D], f32, kind="ExternalOutput").ap()

    from contextlib import ExitStack

    with tile.TileContext(nc) as tc, ExitStack() as ctx:
        consts = ctx.enter_context(tc.tile_pool(name="consts", bufs=1))
        persist = ctx.enter_context(tc.tile_pool(name="persist", bufs=1))
        xkpool = ctx.enter_context(tc.tile_pool(name="xkpool", bufs=KD))
        xvpool = ctx.enter_context(tc.tile_pool(name="xvpool", bufs=KD))
        xqpool = ctx.enter_context(tc.tile_pool(name="xqpool", bufs=KD))
        epool = ctx.enter_context(tc.tile_pool(name="epool", bufs=8))
        rzpool = ctx.enter_context(tc.tile_pool(name="rzpool", bufs=2))
        ospool = ctx.enter_context(tc.tile_pool(name="ospool", bufs=3))
        scp = ctx.enter_context(tc.tile_pool(name="scp", bufs=2, space="PSUM"))
        chp = ctx.enter_context(tc.tile_pool(name="chp", bufs=2, space="PSUM"))
        fillp = ctx.enter_context(tc.tile_pool(name="fillp", bufs=2, space="PSUM"))

        # ---- DMA order: wk, xk, wq, xq[sh0], wv, xv, xq[sh1..], wo ----
        wk_sb = consts.tile([P, KD, 256], bf16, name="wk_sb")
        nc.sync.dma_start(out=wk_sb, in_=wk.rearrange("(k p) e -> p k e", p=P))
        bk_sb = consts.tile([P, 2], f32, name="bk_sb")
        nc.sync.dma_start(out=bk_sb, in_=bk2)

        xk3 = xk.rearrange("(k p) s -> k p s", p=P)
        xk_tiles = []
        for k in range(KD):
            t_ = xkpool.tile([P, s], bf16, tag="xk", name=f"xk{k}")
            nc.sync.dma_start(out=t_, in_=xk3[k])
            xk_tiles.append(t_)

        wq_sb = consts.tile([P, KD, 256], bf16, name="wq_sb")
        nc.sync.dma_start(out=wq_sb, in_=wq.rearrange("(k p) e -> p k e", p=P))
        bq_sb = consts.tile([P, 2], f32, name="bq_sb")
        nc.sync.dma_start(out=bq_sb, in_=bq2)

        xq3 = xq.rearrange("(k p) s -> k p s", p=P)
        xq0_tiles = []  # sh0 quarters: small dedicated tiles (fast lead-in)
        for k in range(KD):
            t_ = xqpool.tile([P, WSC], bf16, tag="xq", name=f"xq{k}")
            nc.sync.dma_start(out=t_, in_=xq3[k][:, 0:WSC])
            xq0_tiles.append(t_)

        wv_sb = consts.tile([P, KD, 256], bf16, name="wv_sb")
        nc.sync.dma_start(out=wv_sb, in_=wv.rearrange("(k p) e -> p k e", p=P))
        bv_sb = consts.tile([P, 256], f32, name="bv_sb")
        nc.sync.dma_start(out=bv_sb, in_=bv4)

        xv3 = xv.rearrange("(k p) s -> k p s", p=P)
        xv_tiles = []
        for k in range(KD):
            t_ = xvpool.tile([P, s], bf16, tag="xv", name=f"xv{k}")
            nc.sync.dma_start(out=t_, in_=xv3[k])
            xv_tiles.append(t_)

        # xq sh1..3 land in the xk pool ring (k-proj is done with those
        # buffers by then; the ring reuse makes the DMA wait for it)
        xq1_tiles = []
        for k in range(KD):
            t_ = xkpool.tile([P, NSH - 1, WSC], bf16, tag="xk", name=f"xq1_{k}")
            for sh in range(1, NSH):
                nc.sync.dma_start(
                    out=t_[:, sh - 1, :],
                    in_=xq3[k][:, sh * WSC:(sh + 1) * WSC],
                )
            xq1_tiles.append(t_)

        def xq_get(k, sh):
            return xq0_tiles[k] if sh == 0 else xq1_tiles[k][:, sh - 1, :]

        wo_sb = consts.tile([P, 2, D], f32r, name="wo_sb")
        nc.sync.dma_start(out=wo_sb, in_=wo)

        # ---- persistent activations -----------------------------------
        q2 = persist.tile([P, 2, s], bf16, name="q2")
        k2 = persist.tile([P, 2, s], bf16, name="k2")
        v_aug = persist.tile([P, NH, NT, 65], bf16, name="v_aug")
        nc.vector.memset(v_aug, 1.0)  # col 64 stays 1.0 = Z ones column
        # attn2: pair-packed normalized attention [128(e of 2 heads), 2, s]
        attn2 = persist.tile([P, 2, s], f32r, name="attn2")

        # ---- helpers (PE work runs in the fill PSUM pool) -------------
        def qk_proj(xget, w_sb, b_sb, dst, p, sh):
            # dst[:, p, sh-block] = (x @ W_pair.T)^T + bias  for 512 cols
            ps = fillp.tile([P, WSC], f32, tag="fill", name="qkps")
            for k in range(KD):
                nc.tensor.matmul(
                    ps,
                    lhsT=w_sb[:, k, p * P:(p + 1) * P],
                    rhs=xget(k, sh),
                    start=(k == 0),
                    stop=(k == KD - 1),
                )
            nc.vector.tensor_scalar(
                dst[:, p, sh * WSC:(sh + 1) * WSC], ps, b_sb[:, p:p + 1],
                None, ALU.add,
            )

        def v_proj(t):
            ps = fillp.tile([P, WSC], f32, tag="fill", name="vps")
            for k in range(KD):
                nc.tensor.matmul(
                    ps[:, 0:256],
                    lhsT=xv_tiles[k][:, t * P:(t + 1) * P],
                    rhs=wv_sb[:, k, :],
                    start=(k == 0),
                    stop=(k == KD - 1),
                )
            for h in range(NH):
                nc.vector.tensor_tensor(
                    v_aug[:, h, t, 0:64],
                    ps[:, h * 64:(h + 1) * 64],
                    bv_sb[:, h * 64:(h + 1) * 64],
                    ALU.add,
                )

        def out_chunk(ci):
            # out rows [ci*128, (ci+1)*128) ; contract attn2 over both pairs
            ob = ospool.tile([P, D], f32, tag="ob", name="ob")
            for nh_i in range(2):
                op = fillp.tile([P, WSC], f32, tag="fill", name="op")
                for p in range(2):
                    nc.tensor.matmul(
                        op,
                        lhsT=attn2[:, p, ci * P:(ci + 1) * P],
                        rhs=wo_sb[:, p, nh_i * 512:(nh_i + 1) * 512],
                        start=(p == 0),
                        stop=(p == 1),
                    )
                nc.vector.tensor_copy(ob[:, nh_i * 512:(nh_i + 1) * 512], op)
            nc.sync.dma_start(out=out[ci * P:(ci + 1) * P, :], in_=ob)

        def normalize(p, sh, ch0, ch1):
            soff = sh * WSC
            # drain chains to SBUF first: frees both chain banks after two
            # quick DVE copies so the next round's attn never waits long
            araw = rzpool.tile([P, 2, WSC], f32, tag="araw", name="araw")
            nc.vector.tensor_copy(araw[0:65, 0, :], ch0[0:65, :])
            nc.vector.tensor_copy(araw[0:65, 1, :], ch1[0:65, :])
            # Z (row 64): partition 64 -> 0 shift via gpsimd-issued DMA,
            # then broadcast (partition_broadcast needs a partition-0 src)
            z0 = rzpool.tile([1, 2, WSC], f32, tag="z0", name="z0")
            nc.gpsimd.dma_start(out=z0, in_=araw[64:65])
            rz = rzpool.tile([64, 2, WSC], f32, tag="rz", name="rz")
            nc.gpsimd.partition_broadcast(rz, z0)
            nc.vector.reciprocal_approx_fast(rz, rz)
            # even head of pair -> attn2 rows 0:64 directly
            nc.vector.tensor_tensor(
                attn2[0:64, p, soff:soff + WSC],
                araw[0:64, 0, :],
                rz[:, 0, :],
                ALU.mult,
            )
            # odd head: scale to tmp then DMA-shift to rows 64:128
            atmp = rzpool.tile([HD, WSC], f32r, tag="atmp", name="atmp")
            nc.vector.tensor_tensor(atmp, araw[0:64, 1, :], rz[:, 1, :], ALU.mult)
            nc.gpsimd.dma_start(
                out=attn2[64:128, p, soff:soff + WSC], in_=atmp
            )

        # ---- lead-in: k-proj (k-DMA paced), q(sh0) --------------------
        for p in range(2):
            for sh in range(NSH):
                qk_proj(
                    lambda k, sh_: xk_tiles[k][:, sh_ * WSC:(sh_ + 1) * WSC],
                    wk_sb, bk_sb, k2, p, sh,
                )
        for p in range(2):
            qk_proj(xq_get, wq_sb, bq_sb, q2, p, 0)

        # ---- filler schedule ------------------------------------------
        # round r = sh*2 + p ; out-proj for sh needs rounds sh*2, sh*2+1
        # normalized, so its 4 chunks spread over rounds sh*2+2, sh*2+3.
        fillers = {r: {} for r in range(2 * NSH)}

        def add_filler(r, sl, job):
            fillers[r].setdefault(sl, []).append(job)

        # v-proj: round-0 filler; xv lands a few slots into round 0, so
        # start at slot 4 (attn lag 6 there keeps the score stream ahead)
        for t in range(NT):
            add_filler(0, max(4, t), lambda t=t: v_proj(t))
        qjobs = [(sh, p) for sh in range(1, NSH) for p in range(2)]
        qslots = [(1, 0), (1, 8), (2, 0), (2, 8), (3, 0), (4, 0)]
        for (r, sl), (sh, p) in zip(qslots, qjobs):
            add_filler(r, sl, lambda sh=sh, p=p: qk_proj(
                xq_get, wq_sb, bq_sb, q2, p, sh))
        for sh in range(NSH - 1):
            for j in range(4):
                r = sh * 2 + 2 + j // 2
                sl = 4 + 8 * (j % 2)
                add_filler(r, sl, lambda ci=sh * 4 + j: out_chunk(ci))

        # ---- attention rounds -----------------------------------------
        for r in range(2 * NSH):
            sh, p = r // 2, r % 2
            soff = sh * WSC
            heads = (2 * p, 2 * p + 1)
            lag = 6 if r == 0 else 2
            ch0 = chp.tile([P, WSC], f32, tag="ch", name="ch0")
            ch1 = chp.tile([P, WSC], f32, tag="ch", name="ch1")
            ets = {}

            def attn_step(t):
                et = ets.pop(t)
                nc.tensor.matmul(
                    ch0[0:65, :],
                    lhsT=v_aug[:, heads[0], t, :],
                    rhs=et[:, 0:WSC],
                    start=(t == 0),
                    stop=(t == NT - 1),
                )
                nc.tensor.matmul(
                    ch1[0:65, :],
                    lhsT=v_aug[:, heads[1], t, :],
                    rhs=et[:, WSC:2 * WSC],
                    start=(t == 0),
                    stop=(t == NT - 1),
                )

            for t in range(NT):
                for job in fillers[r].get(t, []):
                    job()
                sc = scp.tile([P, 2 * WSC], f32, tag="sc", name="sc")
                for hi in range(2):
                    rlo, rhi = (0, 64) if hi == 0 else (64, 128)
                    nc.tensor.matmul(
                        sc[:, hi * WSC:(hi + 1) * WSC],
                        lhsT=k2[rlo:rhi, p, t * P:(t + 1) * P],
                        rhs=q2[rlo:rhi, p, soff:soff + WSC],
                        start=True,
                        stop=True,
                        tile_position=(rlo, 0),
                    )
                et = epool.tile([P, 2 * WSC], bf16, tag="exp", name="et")
                nc.scalar.activation(et, sc, AF.Exp, bias=0.0, scale=0.125)
                ets[t] = et
                if t >= lag:
                    attn_step(t - lag)
            for t in range(NT - lag, NT):
                attn_step(t)
            normalize(p, sh, ch0, ch1)

        # ---- tail: last sh out-proj chunks ----------------------------
        for j in range(4):
            out_chunk((NSH - 1) * 4 + j)

    nc.compile()
    return nc


def get_bass(s=S):
    if s not in _BUILD_CACHE:
        _BUILD_CACHE[s] = build_bass(s)
    return _BUILD_CACHE[s]


def make_in_maps(query, key, value, Wq, bq, Wk, bk, Wv, bv, Wo):
    """Host-side sharding: per-core input dict for core = b*4 + g."""
    in_maps = []
    for core in range(8):
        b, g = core // 4, core % 4
        cs = slice(g * 256, (g + 1) * 256)
        # pair-packed: wo_h[hd + 64*(h%2), h//2, :] = Wo[:, g*256 + h*64 + hd]
        wo_h = (
            np.ascontiguousarray(Wo[:, cs].T)  # [256(h*64+hd), 1024]
            .reshape(2, P, D)
            .transpose(1, 0, 2)
        )
        m = {
            "xq_t": np.ascontiguousarray(query[:, b, :].T).astype(BF16),
            "xk_t": np.ascontiguousarray(key[:, b, :].T).astype(BF16),
            "xv_t": np.ascontiguousarray(value[:, b, :].T).astype(BF16),
            "wq_t": np.ascontiguousarray(Wq[cs, :].T).astype(BF16),
            "wk_t": np.ascontiguousarray(Wk[cs, :].T).astype(BF16),
            "wv_t": np.ascontiguousarray(Wv[cs, :].T).astype(BF16),
            "wo_h": np.ascontiguousarray(wo_h).astype(np.float32),
            "bq2": np.ascontiguousarray(bq[cs].reshape(2, P).T).astype(np.float32),
            "bk2": np.ascontiguousarray(bk[cs].reshape(2, P).T).astype(np.float32),
            "bv4": np.ascontiguousarray(
                np.broadcast_to(bv[cs], (P, 256))
            ).astype(np.float32),
        }
        in_maps.append(m)
    return in_maps


def kernel(query, key, value, Wq, bq, Wk, bk, Wv, bv, Wo, bo):
    from concourse.bass_utils import run_bass_kernel_spmd

    query = np.asarray(query, dtype=np.float32)
    key = np.asarray(key, dtype=np.float32)
    value = np.asarray(value, dtype=np.float32)
    Wq = np.asarray(Wq, dtype=np.float32)
    Wk = np.asarray(Wk, dtype=np.float32)
    Wv = np.asarray(Wv, dtype=np.float32)
    Wo = np.asarray(Wo, dtype=np.float32)

    nc = get_bass(S)
    in_maps = make_in_maps(query, key, value, Wq, bq, Wk, bk, Wv, bv, Wo)
    res = run_bass_kernel_spmd(nc, in_maps, core_ids=list(range(8)))
    outs = [res.results[c]["out"] for c in range(8)]

    full = np.empty((S, B, D), dtype=np.float32)
    bo32 = np.asarray(bo, dtype=np.float32)
    for b in range(B):
        acc = outs[b * 4].astype(np.float32).copy()
        for g in range(1, 4):
            acc += outs[b * 4 + g]
        full[:, b, :] = acc + bo32[None, :]
    return full


# revision 19
# speedup vs baseline: 1.2241x; 1.0107x over previous
"""Multi-head attention (S=2048, B=2, D=1024, H=16, Hd=64) on 8 trn2 cores.

Sharding: core = (batch b, head-group g of 4 heads)  -> 2*4 = 8 cores.
Each core computes the full attention for its 4 heads / 1 batch and a
partial output projection (row-parallel Wo); the host sums the 4 partials
per batch and adds bo.

Schedule (v3): software-pipelined around the ACT engine's exp wall.
  - 8 attention rounds of (sh in 4 s-blocks of 512, p in 2 head-pairs);
    per t-step the PE does 2 score mms (row-paired heads at tile_position
    0/64) + 2 attn chain mms (emitted with lag 2 so chain-buffer reuse
    stalls never block the score stream); ACT does one exp over
    [128, 1024] (both heads packed side by side in one PSUM score tile).
  - PSUM: scores 2x[128,1024] (4 banks) + chains 2x[128,512] (2 banks)
    + fill pool 2x[128,512] (2 banks) for proj/out-proj work that is
    interleaved into the rounds as PE filler (keeps the PE p-state up).
  - DMA order: wk, xk, wq, xq[sh0], wv, xv, xq[sh1..3], wo - so the
    k-projection starts as soon as the first xk tile lands and round 0
    starts right after q2[sh0]; the v-projection runs as round-0 filler
    (attn lag 4 there so it never blocks the score stream).
  - normalize: chains are drained to SBUF immediately (frees the chain
    PSUM bank for the next round after one DVE copy); Z goes partition
    64 -> 0 via a small gpsimd-issued SBUF DMA, then gpsimd
    partition_broadcast (which only honors partition-0 sources), DVE
    reciprocal and the scaling multiplies - all off the PE critical path.
  - out-proj is chunked per 128 output rows and interleaved as filler;
    each chunk DMAs out immediately from the SP queue.
"""

import sys

for _p in ("/opt/trn_rl_repo", "/root/.axon_site/_ro/trn_rl_repo"):
    if _p not in sys.path:
        sys.path.insert(0, _p)

import numpy as np
import ml_dtypes

S = 2048
B = 2
D = 1024
H = 16
HD = 64
NH = 4  # heads per core
P = 128
KD = D // P  # 8 contraction tiles for projections
NT = S // P  # 16 t tiles
WSC = 512  # s-columns per round
NSH = S // WSC  # 4 s-blocks

BF16 = ml_dtypes.bfloat16

_BUILD_CACHE = {}


def build_bass(s=S):
    """Build the per-core Bass module (same program for all 8 cores)."""
    import concourse.bacc as bacc
    import concourse.bass as bass
    import concourse.mybir as mybir
    import concourse.tile as tile

    f32 = mybir.dt.float32
    f32r = mybir.dt.float32r
    bf16 = mybir.dt.bfloat16
    AF = mybir.ActivationFunctionType
    ALU = mybir.AluOpType

    nc = bacc.Bacc("TRN2", target_bir_lowering=False, debug=False, num_devices=8)

    xq = nc.dram_tensor("xq_t", [D, s], bf16, kind="ExternalInput").ap()
    xk = nc.dram_tensor("xk_t", [D, s], bf16, kind="ExternalInput").ap()
    xv = nc.dram_tensor("xv_t", [D, s], bf16, kind="ExternalInput").ap()
    wq = nc.dram_tensor("wq_t", [D, 256], bf16, kind="ExternalInput").ap()
    wk = nc.dram_tensor("wk_t", [D, 256], bf16, kind="ExternalInput").ap()
    wv = nc.dram_tensor("wv_t", [D, 256], bf16, kind="ExternalInput").ap()
    wo = nc.dram_tensor("wo_h", [P, 2, D], bf16, kind="ExternalInput").ap()
    bq2 = nc.dram_tensor("bq2", [P, 2], f32, kind="ExternalInput").ap()
    bk2 = nc.dram_tensor("bk2", [P, 2], f32, kind="ExternalInput").ap()
    bv4 = nc.dram_tensor("bv4", [P, 256], f32, kind="ExternalInput").ap()
    out = nc.dram_tensor("out", [s, D], f32, kind="ExternalOutput").ap()

    from contextlib import ExitStack

    with tile.TileContext(nc) as tc, ExitStack() as ctx:
        consts = ctx.enter_context(tc.tile_pool(name="consts", bufs=1))
        persist = ctx.enter_context(tc.tile_pool(name="persist", bufs=1))
        xkpool = ctx.enter_context(tc.tile_pool(name="xkpool", bufs=2))
        xvpool = ctx.enter_context(tc.tile_pool(name="xvpool", bufs=1))
        xqpool = ctx.enter_context(tc.tile_pool(name="xqpool", bufs=1))
        epool = ctx.enter_context(tc.tile_pool(name="epool", bufs=8))
        rzpool = ctx.enter_context(tc.tile_pool(name="rzpool", bufs=2))
        ospool = ctx.enter_context(tc.tile_pool(name="ospool", bufs=3))
        scp = ctx.enter_context(tc.tile_pool(name="scp", bufs=2, space="PSUM"))
        chp = ctx.enter_context(tc.tile_pool(name="chp", bufs=2, space="PSUM"))
        fillp = ctx.enter_context(tc.tile_pool(name="fillp", bufs=2, space="PSUM"))

        # ---- DMA order: wk, xk, wq, xq[sh0], wv, xv, xq[sh1..], wo ----
        wk_sb = consts.tile([P, KD, 256], bf16, name="wk_sb")
        nc.sync.dma_start(out=wk_sb, in_=wk.rearrange("(k p) e -> p k e", p=P))
        bk_sb = consts.tile([P, 2], f32, name="bk_sb")
        nc.sync.dma_start(out=bk_sb, in_=bk2)

        # few, large DMAs: each dma_start costs ~0.7us of serial SP issue
        # time, so batch k-tiles into halves / single transfers
        xk3 = xk.rearrange("(k p) s -> p k s", p=P)
        xk_halves = []
        for h in range(2):
            t_ = xkpool.tile([P, KD // 2, s], bf16, tag="xk", name=f"xk{h}")
            nc.sync.dma_start(out=t_, in_=xk3[:, h * 4:(h + 1) * 4, :])
            xk_halves.append(t_)

        wq_sb = consts.tile([P, KD, 256], bf16, name="wq_sb")
        nc.sync.dma_start(out=wq_sb, in_=wq.rearrange("(k p) e -> p k e", p=P))
        bq_sb = consts.tile([P, 2], f32, name="bq_sb")
        nc.sync.dma_start(out=bq_sb, in_=bq2)

        xq3 = xq.rearrange("(k p) s -> p k s", p=P)
        xq0_tile = xqpool.tile([P, KD, WSC], bf16, tag="xq0", name="xq0")
        nc.sync.dma_start(out=xq0_tile, in_=xq3[:, :, 0:WSC])

        wv_sb = consts.tile([P, KD, 256], bf16, name="wv_sb")
        nc.sync.dma_start(out=wv_sb, in_=wv.rearrange("(k p) e -> p k e", p=P))
        bv_sb = consts.tile([P, 256], f32, name="bv_sb")
        nc.sync.dma_start(out=bv_sb, in_=bv4)

        xv_tile = xvpool.tile([P, KD, s], bf16, tag="xv", name="xv")
        nc.sync.dma_start(out=xv_tile, in_=xv.rearrange("(k p) s -> p k s", p=P))

        xq1_tile = xqpool.tile([P, KD, NSH - 1, WSC], bf16, tag="xq1", name="xq1")
        nc.sync.dma_start(out=xq1_tile, in_=xq3[:, :, WSC:])

        def xq_get(k, sh):
            return xq0_tile[:, k, :] if sh == 0 else xq1_tile[:, k, sh - 1, :]

        wo_sb = consts.tile([P, 2, D], bf16, name="wo_sb")
        nc.sync.dma_start(out=wo_sb, in_=wo)

        # ---- persistent activations -----------------------------------
        q2 = persist.tile([P, 2, s], bf16, name="q2")
        k2 = persist.tile([P, 2, s], bf16, name="k2")
        v_aug = persist.tile([P, NH, NT, 65], bf16, name="v_aug")
        nc.vector.memset(v_aug, 1.0)  # col 64 stays 1.0 = Z ones column
        # attn2: pair-packed normalized attention [128(e of 2 heads), 2, s]
        attn2 = persist.tile([P, 2, s], bf16, name="attn2")

        # ---- helpers (PE work runs in the fill PSUM pool) -------------
        def qk_proj(xget, w_sb, b_sb, dst, p, sh):
            # dst[:, p, sh-block] = (x @ W_pair.T)^T + bias  for 512 cols
            ps = fillp.tile([P, WSC], f32, tag="fill", name="qkps")
            for k in range(KD):
                nc.tensor.matmul(
                    ps,
                    lhsT=w_sb[:, k, p * P:(p + 1) * P],
                    rhs=xget(k, sh),
                    start=(k == 0),
                    stop=(k == KD - 1),
                )
            nc.vector.tensor_scalar(
                dst[:, p, sh * WSC:(sh + 1) * WSC], ps, b_sb[:, p:p + 1],
                None, ALU.add,
            )

        def v_proj(t):
            ps = fillp.tile([P, WSC], f32, tag="fill", name="vps")
            for k in range(KD):
                nc.tensor.matmul(
                    ps[:, 0:256],
                    lhsT=xv_tile[:, k, t * P:(t + 1) * P],
                    rhs=wv_sb[:, k, :],
                    start=(k == 0),
                    stop=(k == KD - 1),
                )
            for h in range(NH):
                nc.vector.tensor_tensor(
                    v_aug[:, h, t, 0:64],
                    ps[:, h * 64:(h + 1) * 64],
                    bv_sb[:, h * 64:(h + 1) * 64],
                    ALU.add,
                )

        def out_chunk(ci):
            # out rows [ci*128, (ci+1)*128) ; contract attn2 over both pairs
            ob = ospool.tile([P, D], f32, tag="ob", name="ob")
            for nh_i in range(2):
                op = fillp.tile([P, WSC], f32, tag="fill", name="op")
                for p in range(2):
                    nc.tensor.matmul(
                        op,
                        lhsT=attn2[:, p, ci * P:(ci + 1) * P],
                        rhs=wo_sb[:, p, nh_i * 512:(nh_i + 1) * 512],
                        start=(p == 0),
                        stop=(p == 1),
                    )
                nc.vector.tensor_copy(ob[:, nh_i * 512:(nh_i + 1) * 512], op)
            nc.sync.dma_start(out=out[ci * P:(ci + 1) * P, :], in_=ob)

        def normalize(p, sh, ch0, ch1):
            soff = sh * WSC
            # drain chains to SBUF first: frees both chain banks after two
            # quick DVE copies so the next round's attn never waits long
            araw = rzpool.tile([P, 2, WSC], f32, tag="araw", name="araw")
            nc.vector.tensor_copy(araw[0:65, 0, :], ch0[0:65, :])
            nc.vector.tensor_copy(araw[0:65, 1, :], ch1[0:65, :])
            # Z (row 64): partition 64 -> 0 shift via gpsimd-issued DMA,
            # then broadcast (partition_broadcast needs a partition-0 src)
            z0 = rzpool.tile([1, 2, WSC], f32, tag="z0", name="z0")
            nc.sync.dma_start(out=z0, in_=araw[64:65])
            rz = rzpool.tile([64, 2, WSC], f32, tag="rz", name="rz")
            nc.gpsimd.partition_broadcast(rz, z0)
            nc.vector.reciprocal_approx_fast(rz, rz)
            # even head of pair -> attn2 rows 0:64 directly
            nc.vector.tensor_tensor(
                attn2[0:64, p, soff:soff + WSC],
                araw[0:64, 0, :],
                rz[:, 0, :],
                ALU.mult,
            )
            # odd head: scale to tmp then DMA-shift to rows 64:128
            atmp = rzpool.tile([HD, WSC], bf16, tag="atmp", name="atmp")
            nc.vector.tensor_tensor(atmp, araw[0:64, 1, :], rz[:, 1, :], ALU.mult)
            nc.sync.dma_start(
                out=attn2[64:128, p, soff:soff + WSC], in_=atmp
            )

        # ---- lead-in: k-proj (xk-half paced), q(sh0) ------------------
        def xk_get(k, sh):
            return xk_halves[k // 4][:, k % 4, sh * WSC:(sh + 1) * WSC]

        for p in range(2):
            for sh in range(NSH):
                qk_proj(xk_get, wk_sb, bk_sb, k2, p, sh)
        for p in range(2):
            qk_proj(xq_get, wq_sb, bq_sb, q2, p, 0)

        # ---- filler schedule ------------------------------------------
        # round r = sh*2 + p ; out-proj for sh needs rounds sh*2, sh*2+1
        # normalized, so its 4 chunks spread over rounds sh*2+2, sh*2+3.
        fillers = {r: {} for r in range(2 * NSH)}

        def add_filler(r, sl, job):
            fillers[r].setdefault(sl, []).append(job)

        # v-proj: round-0 filler; xv lands a few slots into round 0, so
        # spread over slots 5..15 (attn lag 6 keeps the score stream ahead)
        for t in range(NT):
            add_filler(0, 5 + (t * 11) // 16, lambda t=t: v_proj(t))
        qjobs = [(sh, p) for sh in range(1, NSH) for p in range(2)]
        qslots = [(1, 0), (1, 8), (2, 0), (3, 0), (4, 0), (4, 8)]
        for (r, sl), (sh, p) in zip(qslots, qjobs):
            add_filler(r, sl, lambda sh=sh, p=p: qk_proj(
                xq_get, wq_sb, bq_sb, q2, p, sh))
        oslots = {0: [(2, 4), (2, 12), (3, 4), (3, 12)],
                  1: [(4, 4), (4, 12), (5, 0), (5, 8)],
                  2: [(6, 0), (6, 8), (7, 0), (7, 8)]}
        for sh, slots in oslots.items():
            for j, (r, sl) in enumerate(slots):
                add_filler(r, sl, lambda ci=sh * 4 + j: out_chunk(ci))

        # ---- attention rounds -----------------------------------------
        for r in range(2 * NSH):
            sh, p = r // 2, r % 2
            soff = sh * WSC
            heads = (2 * p, 2 * p + 1)
            lag = 6 if r == 0 else 2
            ch0 = chp.tile([P, WSC], f32, tag="ch", name="ch0")
            ch1 = chp.tile([P, WSC], f32, tag="ch", name="ch1")
            ets = {}

            def attn_step(t):
                et = ets.pop(t)
                nc.tensor.matmul(
                    ch0[0:65, :],
                    lhsT=v_aug[:, heads[0], t, :],
                    rhs=et[:, 0:WSC],
                    start=(t == 0),
                    stop=(t == NT - 1),
                )
                nc.tensor.matmul(
                    ch1[0:65, :],
                    lhsT=v_aug[:, heads[1], t, :],
                    rhs=et[:, WSC:2 * WSC],
                    start=(t == 0),
                    stop=(t == NT - 1),
                )

            for t in range(NT):
                for job in fillers[r].get(t, []):
                    job()
                sc = scp.tile([P, 2 * WSC], f32, tag="sc", name="sc")
                for hi in range(2):
                    rlo, rhi = (0, 64) if hi == 0 else (64, 128)
                    nc.tensor.matmul(
                        sc[:, hi * WSC:(hi + 1) * WSC],
                        lhsT=k2[rlo:rhi, p, t * P:(t + 1) * P],
                        rhs=q2[rlo:rhi, p, soff:soff + WSC],
                        start=True,
                        stop=True,
                        tile_position=(rlo, 0),
                    )
                et = epool.tile([P, 2 * WSC], bf16, tag="exp", name="et")
                nc.scalar.activation(et, sc, AF.Exp, bias=0.0, scale=0.125)
                ets[t] = et
                if t >= lag:
                    attn_step(t - lag)
            for t in range(NT - lag, NT):
                attn_step(t)
            normalize(p, sh, ch0, ch1)

        # ---- tail: last sh out-proj chunks ----------------------------
        for j in range(4):
            out_chunk((NSH - 1) * 4 + j)

    nc.compile()
    return nc


def get_bass(s=S):
    if s not in _BUILD_CACHE:
        _BUILD_CACHE[s] = build_bass(s)
    return _BUILD_CACHE[s]


def make_in_maps(query, key, value, Wq, bq, Wk, bk, Wv, bv, Wo):
    """Host-side sharding: per-core input dict for core = b*4 + g."""
    in_maps = []
    for core in range(8):
        b, g = core // 4, core % 4
        cs = slice(g * 256, (g + 1) * 256)
        # pair-packed: wo_h[hd + 64*(h%2), h//2, :] = Wo[:, g*256 + h*64 + hd]
        wo_h = (
            np.ascontiguousarray(Wo[:, cs].T)  # [256(h*64+hd), 1024]
            .reshape(2, P, D)
            .transpose(1, 0, 2)
        )
        m = {
            "xq_t": np.ascontiguousarray(query[:, b, :].T).astype(BF16),
            "xk_t": np.ascontiguousarray(key[:, b, :].T).astype(BF16),
            "xv_t": np.ascontiguousarray(value[:, b, :].T).astype(BF16),
            "wq_t": np.ascontiguousarray(Wq[cs, :].T).astype(BF16),
            "wk_t": np.ascontiguousarray(Wk[cs, :].T).astype(BF16),
            "wv_t": np.ascontiguousarray(Wv[cs, :].T).astype(BF16),
            "wo_h": np.ascontiguousarray(wo_h).astype(BF16),
            "bq2": np.ascontiguousarray(bq[cs].reshape(2, P).T).astype(np.float32),
            "bk2": np.ascontiguousarray(bk[cs].reshape(2, P).T).astype(np.float32),
            "bv4": np.ascontiguousarray(
                np.broadcast_to(bv[cs], (P, 256))
            ).astype(np.float32),
        }
        in_maps.append(m)
    return in_maps


def kernel(query, key, value, Wq, bq, Wk, bk, Wv, bv, Wo, bo):
    from concourse.bass_utils import run_bass_kernel_spmd

    query = np.asarray(query, dtype=np.float32)
    key = np.asarray(key, dtype=np.float32)
    value = np.asarray(value, dtype=np.float32)
    Wq = np.asarray(Wq, dtype=np.float32)
    Wk = np.asarray(Wk, dtype=np.float32)
    Wv = np.asarray(Wv, dtype=np.float32)
    Wo = np.asarray(Wo, dtype=np.float32)

    nc = get_bass(S)
    in_maps = make_in_maps(query, key, value, Wq, bq, Wk, bk, Wv, bv, Wo)
    res = run_bass_kernel_spmd(nc, in_maps, core_ids=list(range(8)))
    outs = [res.results[c]["out"] for c in range(8)]

    full = np.empty((S, B, D), dtype=np.float32)
    bo32 = np.asarray(bo, dtype=np.float32)
    for b in range(B):
        acc = outs[b * 4].astype(np.float32).copy()
        for g in range(1, 4):
            acc += outs[b * 4 + g]
        full[:, b, :] = acc + bo32[None, :]
    return full


# revision 24
# speedup vs baseline: 1.2592x; 1.0287x over previous
"""Multi-head attention (S=2048, B=2, D=1024, H=16, Hd=64) on 8 trn2 cores.

Sharding: core = (batch b, head-group g of 4 heads)  -> 2*4 = 8 cores.
Each core computes the full attention for its 4 heads / 1 batch and a
partial output projection (row-parallel Wo); the host sums the 4 partials
per batch and adds bo.

Schedule (v3): software-pipelined around the ACT engine's exp wall.
  - 8 attention rounds of (sh in 4 s-blocks of 512, p in 2 head-pairs);
    per t-step the PE does 2 score mms (row-paired heads at tile_position
    0/64) + 2 attn chain mms (emitted with lag 2 so chain-buffer reuse
    stalls never block the score stream); ACT does one exp over
    [128, 1024] (both heads packed side by side in one PSUM score tile).
  - PSUM: scores 2x[128,1024] (4 banks) + chains 2x[128,512] (2 banks)
    + fill pool 2x[128,512] (2 banks) for proj/out-proj work that is
    interleaved into the rounds as PE filler (keeps the PE p-state up).
  - DMA order: wk, xk, wq, xq[sh0], wv, xv, xq[sh1..3], wo - so the
    k-projection starts as soon as the first xk tile lands and round 0
    starts right after q2[sh0]; the v-projection runs as round-0 filler
    (attn lag 4 there so it never blocks the score stream).
  - normalize: chains are drained to SBUF immediately (frees the chain
    PSUM bank for the next round after one DVE copy); Z goes partition
    64 -> 0 via a small gpsimd-issued SBUF DMA, then gpsimd
    partition_broadcast (which only honors partition-0 sources), DVE
    reciprocal and the scaling multiplies - all off the PE critical path.
  - out-proj is chunked per 128 output rows and interleaved as filler;
    each chunk DMAs out immediately from the SP queue.
"""

import sys

for _p in ("/opt/trn_rl_repo", "/root/.axon_site/_ro/trn_rl_repo"):
    if _p not in sys.path:
        sys.path.insert(0, _p)

import numpy as np
import ml_dtypes

S = 2048
B = 2
D = 1024
H = 16
HD = 64
NH = 4  # heads per core
P = 128
KD = D // P  # 8 contraction tiles for projections
NT = S // P  # 16 t tiles
WSC = 512  # s-columns per round
NSH = S // WSC  # 4 s-blocks

BF16 = ml_dtypes.bfloat16

_BUILD_CACHE = {}


def build_bass(s=S):
    """Build the per-core Bass module (same program for all 8 cores)."""
    import concourse.bacc as bacc
    import concourse.bass as bass
    import concourse.mybir as mybir
    import concourse.tile as tile

    f32 = mybir.dt.float32
    f32r = mybir.dt.float32r
    bf16 = mybir.dt.bfloat16
    AF = mybir.ActivationFunctionType
    ALU = mybir.AluOpType

    nc = bacc.Bacc("TRN2", target_bir_lowering=False, debug=False, num_devices=8)

    xq = nc.dram_tensor("xq_t", [D, s], bf16, kind="ExternalInput").ap()
    xk = nc.dram_tensor("xk_t", [D, s], bf16, kind="ExternalInput").ap()
    xv = nc.dram_tensor("xv_t", [D, s], bf16, kind="ExternalInput").ap()
    wq = nc.dram_tensor("wq_t", [D, 256], bf16, kind="ExternalInput").ap()
    wk = nc.dram_tensor("wk_t", [D, 256], bf16, kind="ExternalInput").ap()
    wv = nc.dram_tensor("wv_t", [D, 256], bf16, kind="ExternalInput").ap()
    wo = nc.dram_tensor("wo_h", [P, 2, D], bf16, kind="ExternalInput").ap()
    bq2 = nc.dram_tensor("bq2", [P, 2], f32, kind="ExternalInput").ap()
    bk2 = nc.dram_tensor("bk2", [P, 2], f32, kind="ExternalInput").ap()
    bv4 = nc.dram_tensor("bv4", [P, 256], f32, kind="ExternalInput").ap()
    out = nc.dram_tensor("out", [s, D], f32, kind="ExternalOutput").ap()

    from contextlib import ExitStack

    with tile.TileContext(nc) as tc, ExitStack() as ctx:
        consts = ctx.enter_context(tc.tile_pool(name="consts", bufs=1))
        persist = ctx.enter_context(tc.tile_pool(name="persist", bufs=1))
        xkpool = ctx.enter_context(tc.tile_pool(name="xkpool", bufs=NSH))
        xvpool = ctx.enter_context(tc.tile_pool(name="xvpool", bufs=1))
        xqpool = ctx.enter_context(tc.tile_pool(name="xqpool", bufs=1))
        epool = ctx.enter_context(tc.tile_pool(name="epool", bufs=8))
        rzpool = ctx.enter_context(tc.tile_pool(name="rzpool", bufs=2))
        ospool = ctx.enter_context(tc.tile_pool(name="ospool", bufs=3))
        scp = ctx.enter_context(tc.tile_pool(name="scp", bufs=2, space="PSUM"))
        chp = ctx.enter_context(tc.tile_pool(name="chp", bufs=2, space="PSUM"))
        fillp = ctx.enter_context(tc.tile_pool(name="fillp", bufs=2, space="PSUM"))

        # ---- DMA order: wk, xk, wq, xq[sh0], wv, xv, xq[sh1..], wo ----
        wk_sb = consts.tile([P, KD, 256], bf16, name="wk_sb")
        nc.sync.dma_start(out=wk_sb, in_=wk.rearrange("(k p) e -> p k e", p=P))
        bk_sb = consts.tile([P, 2], f32, name="bk_sb")
        nc.sync.dma_start(out=bk_sb, in_=bk2)

        # few, large DMAs: each dma_start costs ~0.7us of serial SP issue
        # time. xk lands in four 512-column blocks so the first k-proj
        # chain (which contracts all k but only needs 512 s-columns)
        # starts as soon as block 0 arrives.
        xk3 = xk.rearrange("(k p) s -> p k s", p=P)
        xk_blocks = []
        for sh in range(NSH):
            t_ = xkpool.tile([P, KD, WSC], bf16, tag="xk", name=f"xk{sh}")
            nc.sync.dma_start(out=t_, in_=xk3[:, :, sh * WSC:(sh + 1) * WSC])
            xk_blocks.append(t_)

        wq_sb = consts.tile([P, KD, 256], bf16, name="wq_sb")
        nc.sync.dma_start(out=wq_sb, in_=wq.rearrange("(k p) e -> p k e", p=P))
        bq_sb = consts.tile([P, 2], f32, name="bq_sb")
        nc.sync.dma_start(out=bq_sb, in_=bq2)

        xq3 = xq.rearrange("(k p) s -> p k s", p=P)
        xq0_tile = xqpool.tile([P, KD, WSC], bf16, tag="xq0", name="xq0")
        nc.sync.dma_start(out=xq0_tile, in_=xq3[:, :, 0:WSC])

        wv_sb = consts.tile([P, KD, 256], bf16, name="wv_sb")
        nc.sync.dma_start(out=wv_sb, in_=wv.rearrange("(k p) e -> p k e", p=P))
        bv_sb = consts.tile([P, 256], f32, name="bv_sb")
        nc.sync.dma_start(out=bv_sb, in_=bv4)

        xv_tile = xvpool.tile([P, KD, s], bf16, tag="xv", name="xv")
        nc.sync.dma_start(out=xv_tile, in_=xv.rearrange("(k p) s -> p k s", p=P))

        xq1_tile = xqpool.tile([P, KD, NSH - 1, WSC], bf16, tag="xq1", name="xq1")
        nc.sync.dma_start(out=xq1_tile, in_=xq3[:, :, WSC:])

        def xq_get(k, sh):
            return xq0_tile[:, k, :] if sh == 0 else xq1_tile[:, k, sh - 1, :]

        wo_sb = consts.tile([P, 2, D], bf16, name="wo_sb")
        nc.sync.dma_start(out=wo_sb, in_=wo)

        # ---- persistent activations -----------------------------------
        q2 = persist.tile([P, 2, s], bf16, name="q2")
        k2 = persist.tile([P, 2, s], bf16, name="k2")
        v_aug = persist.tile([P, NH, NT, 65], bf16, name="v_aug")
        nc.vector.memset(v_aug, 1.0)  # col 64 stays 1.0 = Z ones column
        # attn2: pair-packed normalized attention [128(e of 2 heads), 2, s]
        attn2 = persist.tile([P, 2, s], bf16, name="attn2")

        # ---- helpers (PE work runs in the fill PSUM pool) -------------
        def qk_proj(xget, w_sb, b_sb, dst, p, sh):
            # dst[:, p, sh-block] = (x @ W_pair.T)^T + bias  for 512 cols
            ps = fillp.tile([P, WSC], f32, tag="fill", name="qkps")
            for k in range(KD):
                nc.tensor.matmul(
                    ps,
                    lhsT=w_sb[:, k, p * P:(p + 1) * P],
                    rhs=xget(k, sh),
                    start=(k == 0),
                    stop=(k == KD - 1),
                )
            nc.vector.tensor_scalar(
                dst[:, p, sh * WSC:(sh + 1) * WSC], ps, b_sb[:, p:p + 1],
                None, ALU.add,
            )

        def v_proj(t):
            ps = fillp.tile([P, WSC], f32, tag="fill", name="vps")
            for k in range(KD):
                nc.tensor.matmul(
                    ps[:, 0:256],
                    lhsT=xv_tile[:, k, t * P:(t + 1) * P],
                    rhs=wv_sb[:, k, :],
                    start=(k == 0),
                    stop=(k == KD - 1),
                )
            for h in range(NH):
                nc.vector.tensor_tensor(
                    v_aug[:, h, t, 0:64],
                    ps[:, h * 64:(h + 1) * 64],
                    bv_sb[:, h * 64:(h + 1) * 64],
                    ALU.add,
                )

        def out_chunk(ci):
            # out rows [ci*128, (ci+1)*128) ; contract attn2 over both pairs
            ob = ospool.tile([P, D], f32, tag="ob", name="ob")
            for nh_i in range(2):
                op = fillp.tile([P, WSC], f32, tag="fill", name="op")
                for p in range(2):
                    nc.tensor.matmul(
                        op,
                        lhsT=attn2[:, p, ci * P:(ci + 1) * P],
                        rhs=wo_sb[:, p, nh_i * 512:(nh_i + 1) * 512],
                        start=(p == 0),
                        stop=(p == 1),
                    )
                nc.vector.tensor_copy(ob[:, nh_i * 512:(nh_i + 1) * 512], op)
            nc.sync.dma_start(out=out[ci * P:(ci + 1) * P, :], in_=ob)

        def normalize(p, sh, ch0, ch1):
            soff = sh * WSC
            # drain chains to SBUF first: frees both chain banks after two
            # quick DVE copies so the next round's attn never waits long
            araw = rzpool.tile([P, 2, WSC], f32, tag="araw", name="araw")
            nc.vector.tensor_copy(araw[0:65, 0, :], ch0[0:65, :])
            nc.vector.tensor_copy(araw[0:65, 1, :], ch1[0:65, :])
            # Z (row 64): partition 64 -> 0 shift via gpsimd-issued DMA,
            # then broadcast (partition_broadcast needs a partition-0 src)
            z0 = rzpool.tile([1, 2, WSC], f32, tag="z0", name="z0")
            nc.sync.dma_start(out=z0, in_=araw[64:65])
            rz = rzpool.tile([64, 2, WSC], f32, tag="rz", name="rz")
            nc.gpsimd.partition_broadcast(rz, z0)
            nc.vector.reciprocal_approx_fast(rz, rz)
            # even head of pair -> attn2 rows 0:64 directly
            nc.vector.tensor_tensor(
                attn2[0:64, p, soff:soff + WSC],
                araw[0:64, 0, :],
                rz[:, 0, :],
                ALU.mult,
            )
            # odd head: scale to tmp then DMA-shift to rows 64:128
            atmp = rzpool.tile([HD, WSC], bf16, tag="atmp", name="atmp")
            nc.vector.tensor_tensor(atmp, araw[0:64, 1, :], rz[:, 1, :], ALU.mult)
            nc.sync.dma_start(
                out=attn2[64:128, p, soff:soff + WSC], in_=atmp
            )

        # ---- lead-in: k-proj (xk-block paced, sh-major), q(sh0) -------
        def xk_get(k, sh):
            return xk_blocks[sh][:, k, :]

        for sh in range(NSH):
            for p in range(2):
                qk_proj(xk_get, wk_sb, bk_sb, k2, p, sh)
        for p in range(2):
            qk_proj(xq_get, wq_sb, bq_sb, q2, p, 0)

        # ---- filler schedule ------------------------------------------
        # round r = sh*2 + p ; out-proj for sh needs rounds sh*2, sh*2+1
        # normalized, so its 4 chunks spread over rounds sh*2+2, sh*2+3.
        fillers = {r: {} for r in range(2 * NSH)}

        def add_filler(r, sl, job):
            fillers[r].setdefault(sl, []).append(job)

        # v-proj: round-0 filler; xv lands a few slots into round 0, so
        # spread over slots 5..15 (attn lag 6 keeps the score stream ahead)
        for t in range(NT):
            add_filler(0, 5 + (t * 11) // 16, lambda t=t: v_proj(t))
        qjobs = [(sh, p) for sh in range(1, NSH) for p in range(2)]
        qslots = [(1, 0), (1, 8), (2, 0), (3, 0), (4, 0), (4, 8)]
        for (r, sl), (sh, p) in zip(qslots, qjobs):
            add_filler(r, sl, lambda sh=sh, p=p: qk_proj(
                xq_get, wq_sb, bq_sb, q2, p, sh))
        # NOTE: out_chunk(sh) depends on normalize(sh*2+1), which is
        # emitted at slot 1 of round sh*2+2 - chunks there must sit at
        # slot >= 2 or the RAW dependency is never formed (stale read)
        oslots = {0: [(2, 4), (2, 12), (3, 4), (3, 12)],
                  1: [(4, 4), (4, 12), (5, 0), (5, 8)],
                  2: [(6, 2), (6, 9), (7, 0), (7, 8)]}
        for sh, slots in oslots.items():
            for j, (r, sl) in enumerate(slots):
                add_filler(r, sl, lambda ci=sh * 4 + j: out_chunk(ci))

        # ---- attention rounds -----------------------------------------
        # the previous round's attn-drain + normalize are emitted in the
        # first slots of the next round, so the score/exp stream never
        # waits behind them at a boundary
        pending = []
        for r in range(2 * NSH):
            sh, p = r // 2, r % 2
            soff = sh * WSC
            heads = (2 * p, 2 * p + 1)
            lag = 6 if r == 0 else 2
            ch0 = chp.tile([P, WSC], f32, tag="ch", name="ch0")
            ch1 = chp.tile([P, WSC], f32, tag="ch", name="ch1")
            ets = {}

            def attn_step(t, ch0=ch0, ch1=ch1, heads=heads, ets=ets):
                et = ets.pop(t)
                nc.tensor.matmul(
                    ch0[0:65, :],
                    lhsT=v_aug[:, heads[0], t, :],
                    rhs=et[:, 0:WSC],
                    start=(t == 0),
                    stop=(t == NT - 1),
                )
                nc.tensor.matmul(
                    ch1[0:65, :],
                    lhsT=v_aug[:, heads[1], t, :],
                    rhs=et[:, WSC:2 * WSC],
                    start=(t == 0),
                    stop=(t == NT - 1),
                )

            for t in range(NT):
                sc = scp.tile([P, 2 * WSC], f32, tag="sc", name="sc")
                for hi in range(2):
                    rlo, rhi = (0, 64) if hi == 0 else (64, 128)
                    nc.tensor.matmul(
                        sc[:, hi * WSC:(hi + 1) * WSC],
                        lhsT=k2[rlo:rhi, p, t * P:(t + 1) * P],
                        rhs=q2[rlo:rhi, p, soff:soff + WSC],
                        start=True,
                        stop=True,
                        tile_position=(rlo, 0),
                    )
                et = epool.tile([P, 2 * WSC], bf16, tag="exp", name="et")
                nc.scalar.activation(et, sc, AF.Exp, bias=0.0, scale=0.125)
                ets[t] = et
                if t == 0:  # drain previous round's chains
                    for job in pending[:-1]:
                        job()
                elif t == 1 and pending:
                    pending[-1]()  # previous round's normalize
                for job in fillers[r].get(t, []):
                    job()
                if t >= lag:
                    attn_step(t - lag)
            pending = [
                lambda t=t, f=attn_step: f(t) for t in range(NT - lag, NT)
            ] + [lambda p=p, sh=sh, a=ch0, b=ch1: normalize(p, sh, a, b)]

        # ---- tail: drain last round, then last sh out-proj chunks -----
        for job in pending:
            job()
        for j in range(4):
            out_chunk((NSH - 1) * 4 + j)

    nc.compile()
    return nc


def get_bass(s=S):
    if s not in _BUILD_CACHE:
        _BUILD_CACHE[s] = build_bass(s)
    return _BUILD_CACHE[s]


def make_in_maps(query, key, value, Wq, bq, Wk, bk, Wv, bv, Wo):
    """Host-side sharding: per-core input dict for core = b*4 + g."""
    in_maps = []
    for core in range(8):
        b, g = core // 4, core % 4
        cs = slice(g * 256, (g + 1) * 256)
        # pair-packed: wo_h[hd + 64*(h%2), h//2, :] = Wo[:, g*256 + h*64 + hd]
        wo_h = (
            np.ascontiguousarray(Wo[:, cs].T)  # [256(h*64+hd), 1024]
            .reshape(2, P, D)
            .transpose(1, 0, 2)
        )
        m = {
            "xq_t": np.ascontiguousarray(query[:, b, :].T).astype(BF16),
            "xk_t": np.ascontiguousarray(key[:, b, :].T).astype(BF16),
            "xv_t": np.ascontiguousarray(value[:, b, :].T).astype(BF16),
            "wq_t": np.ascontiguousarray(Wq[cs, :].T).astype(BF16),
            "wk_t": np.ascontiguousarray(Wk[cs, :].T).astype(BF16),
            "wv_t": np.ascontiguousarray(Wv[cs, :].T).astype(BF16),
            "wo_h": np.ascontiguousarray(wo_h).astype(BF16),
            "bq2": np.ascontiguousarray(bq[cs].reshape(2, P).T).astype(np.float32),
            "bk2": np.ascontiguousarray(bk[cs].reshape(2, P).T).astype(np.float32),
            "bv4": np.ascontiguousarray(
                np.broadcast_to(bv[cs], (P, 256))
            ).astype(np.float32),
        }
        in_maps.append(m)
    return in_maps


def kernel(query, key, value, Wq, bq, Wk, bk, Wv, bv, Wo, bo):
    from concourse.bass_utils import run_bass_kernel_spmd

    query = np.asarray(query, dtype=np.float32)
    key = np.asarray(key, dtype=np.float32)
    value = np.asarray(value, dtype=np.float32)
    Wq = np.asarray(Wq, dtype=np.float32)
    Wk = np.asarray(Wk, dtype=np.float32)
    Wv = np.asarray(Wv, dtype=np.float32)
    Wo = np.asarray(Wo, dtype=np.float32)

    nc = get_bass(S)
    in_maps = make_in_maps(query, key, value, Wq, bq, Wk, bk, Wv, bv, Wo)
    res = run_bass_kernel_spmd(nc, in_maps, core_ids=list(range(8)))
    outs = [res.results[c]["out"] for c in range(8)]

    full = np.empty((S, B, D), dtype=np.float32)
    bo32 = np.asarray(bo, dtype=np.float32)
    for b in range(B):
        acc = outs[b * 4].astype(np.float32).copy()
        for g in range(1, 4):
            acc += outs[b * 4 + g]
        full[:, b, :] = acc + bo32[None, :]
    return full
